# revision 25
# baseline (speedup 1.0000x reference)
"""Collaborative attention (nn_CollaborativeAttention) on 8 Trainium2 NeuronCores.

Reference math (B=2, S=2048, D=1024, H=16 heads, head mixing over full DKQ=1024):
    q = h @ Wq.T ; k = h @ Wk.T ; v = h @ Wv.T + bv
    scores[b,h,s,t] = sum_e q[b,s,e] * mixing[h,e] * k[b,t,e] / sqrt(64)
    probs = softmax_t(scores) ; ctx[b,s,:] = concat_h(probs @ v_head)

Sharding: core c handles batch b = c//4 and t-quarter qt = c%4 (512 keys), for
ALL 16 heads and ALL 2048 queries.  Each core emits, per head, the partial
numerator sum_{t in quarter} exp(score)*v_head (64 cols) and the partial
denominator sum_t exp(score) (1 col).  The host sums the 4 partials per batch
and divides — this removes the 4x-redundant kt projection of the old
(batch x head-group) sharding (each core now projects only its own 512 keys)
and shrinks the probs@v matmul from a 128-col-padded moving operand to a
65-col one (stationary loads are free on the PE).

The t-quarter is selected host-side by rolling ht's columns so the quarter
sits at columns 0:512 (the q projection is order-invariant; the host unrolls
the output rows).  No device-side dynamic addressing, no cross-core comms.

Device dataflow (per core, all matmuls f16 with fp32 PSUM accumulation):
    ht [1152,2048]   host-transposed hidden (+ ones row), rolled
    kt[e,t]   = WkT.T @ ht[:, :512]        (e on partitions; 512 keys only)
    qT[e,s]   = WqT.T @ ht                 (all 2048 queries)
    v[t,dva]  = ht[:, :512].T @ WvT_aug    (natural; 16 x (64 v cols + ones))
    per head h:
      mk[e,t]   = kt * mixing[h,e]         (DVE/ACT, per-partition broadcast)
      scoresT   = mk.T @ qT  -> psum[t,s] -> exp(0.125*x) -> expT[t,s] (f16)
      ctx unit (s-chunk): psum[s,65] = sum_tc expT_chunk.T @ v[:, h*65:h*65+65]
      (col 64 = ones -> partial softmax denominator); DVE copy -> DMA out.
"""

import math

import numpy as np

B, S, D = 2, 2048, 1024
H, DV = 16, 1024
N_CORES = 8
DH = 64  # head dim
CW = DH + 1  # output unit width: 64 v cols + denominator col
P = 128
EC = 8  # e-chunks (1024/128)
DC = 9  # d-chunks incl. bias row (1152/128)
TQ = 512  # keys per core (t-quarter)
TC = 4  # t-chunks of 128
NB = 512  # s-block width
SB = 4  # number of s blocks
SC = 4  # s-chunks of 128 per s-block
SCALE = 1.0 / math.sqrt(D / H)  # 0.125

_CACHE: dict = {}


def build_program():
    """Build the (SPMD, per-core) Bass program."""
    import concourse.bass as bass
    import concourse.mybir as mybir
    from concourse import bacc
    from concourse.tile import TileContext

    f32 = mybir.dt.float32
    f16 = mybir.dt.float16
    mult = mybir.AluOpType.mult
    Exp = mybir.ActivationFunctionType.Exp
    Copy = mybir.ActivationFunctionType.Copy

    nc = bacc.Bacc("TRN2", target_bir_lowering=False, debug=True)
    ht = nc.dram_tensor("ht", [DC * P, S], f16, kind="ExternalInput")
    wqt = nc.dram_tensor("wqt", [D, D], f16, kind="ExternalInput")
    wkt = nc.dram_tensor("wkt", [D, D], f16, kind="ExternalInput")
    wvt = nc.dram_tensor("wvt", [DC * P, H * CW], f16, kind="ExternalInput")
    mix = nc.dram_tensor("mix", [P, EC * H], f32, kind="ExternalInput")
    # [head, s-block, partition, s-chunk*col]: contiguous 1040B per partition
    # per (h, sb) piece so each output DMA is a single fat descriptor per row
    ctx_o = nc.dram_tensor("ctx", [H, SB, P, SC * CW], f32, kind="ExternalOutput")

    ht_view = ht.rearrange("(c p) s -> p c s", p=P)  # [128, 9, 2048]
    wqt_view = wqt.rearrange("(c p) e -> p c e", p=P)  # [128, 8, 1024]
    wkt_view = wkt.rearrange("(c p) e -> p c e", p=P)
    wvt_view = wvt.rearrange("(c p) (h w) -> p c h w", p=P, h=H, w=CW)  # [128,9,16,65]
    ctx_view = ctx_o.rearrange("h sb p (sc c) -> h sb p sc c", sc=SC, c=CW)

    with TileContext(nc) as tc:
        with (
            tc.tile_pool(name="const", bufs=1) as cpool,
            tc.tile_pool(name="htp", bufs=1) as htpool,
            tc.tile_pool(name="mkp", bufs=2) as mkpool,
            tc.tile_pool(name="expt", bufs=12) as epool,
            tc.tile_pool(name="outp", bufs=2) as opool,
            tc.tile_pool(name="psm", bufs=4, space="PSUM") as psm,
            tc.tile_pool(name="psc", bufs=4, space="PSUM") as psc,
        ):
            # ---- input DMAs (two issue queues: SP and gpsimd/SWDGE) ----
            w_k = cpool.tile([P, EC, D], f16, tag="wk")
            hta = htpool.tile([P, DC, S], f16, tag="hta")
            # first chunk split so the PE's first matmul starts sooner
            nc.sync.dma_start(w_k[:, 0, 0:NB], wkt_view[:, 0, 0:NB])
            nc.gpsimd.dma_start(hta[:, 0, 0:TQ], ht_view[:, 0, 0:TQ])
            nc.sync.dma_start(w_k[:, 0, NB:D], wkt_view[:, 0, NB:D])
            for d in range(1, EC):
                nc.sync.dma_start(w_k[:, d, :], wkt_view[:, d, :])
                nc.gpsimd.dma_start(hta[:, d, 0:TQ], ht_view[:, d, 0:TQ])
            w_q = cpool.tile([P, EC, D], f16, tag="wq")
            for d in range(EC):
                nc.sync.dma_start(w_q[:, d, :], wqt_view[:, d, :])
            for d in range(DC):
                nc.gpsimd.dma_start(hta[:, d, TQ:S], ht_view[:, d, TQ:S])
            nc.gpsimd.dma_start(hta[:, EC, 0:TQ], ht_view[:, EC, 0:TQ])
            w_v = cpool.tile([P, DC, H, CW], f16, tag="wv")
            for d in range(DC):
                nc.sync.dma_start(w_v[:, d], wvt_view[:, d])
            mx = cpool.tile([P, EC * H], f32, tag="mx")
            nc.sync.dma_start(mx[:], mix[:])

            # ---- phase 1: kt (t-quarter), qT (all s), v (t-quarter) ----
            # kt and q(sb0) run d-outer with 8 concurrent PSUM groups so the
            # PE consumes each arriving wk/wq/ht chunk immediately instead of
            # being paced by the 8-chunk DMA chain of a single d-inner group.
            def proj8(w, cols, copy_to):
                pss = [
                    (psm if e % 2 == 0 else psc).tile(
                        [P, NB], f32, tag="m" if e % 2 == 0 else "c",
                        name=f"pj_{e}",
                    )
                    for e in range(EC)
                ]
                for d in range(EC):
                    for e in range(EC):
                        nc.tensor.matmul(
                            pss[e],
                            w[:, d, e * P : (e + 1) * P],
                            hta[:, d, cols],
                            start=(d == 0),
                            stop=(d == EC - 1),
                        )
                for e in range(EC):
                    nc.vector.tensor_copy(copy_to[e], pss[e])

            ktt = cpool.tile([P, EC, TQ], f16, tag="kt")
            proj8(w_k, slice(0, TQ), [ktt[:, e, :] for e in range(EC)])

            qt = cpool.tile([P, EC, S], f16, tag="qt")
            proj8(w_q, slice(0, NB), [qt[:, e, 0:NB] for e in range(EC)])
            for sb in range(1, SB):
                for e in range(EC):
                    ps = psm.tile([P, NB], f32, tag="m")
                    for d in range(EC):
                        nc.tensor.matmul(
                            ps,
                            w_q[:, d, e * P : (e + 1) * P],
                            hta[:, d, sb * NB : (sb + 1) * NB],
                            start=(d == 0),
                            stop=(d == EC - 1),
                        )
                    nc.vector.tensor_copy(qt[:, e, sb * NB : (sb + 1) * NB], ps)

            # v: project only the 16x64 value columns (the per-head ones
            # column that yields the softmax denominator is just memset)
            vt = cpool.tile([P, TC, H, CW], f16, tag="vt")
            nc.vector.memset(vt[:, :, :, DH], 1.0)
            for tcc in range(TC):
                for g in range(2):
                    ps = psm.tile([P, 8, DH], f32, tag="m")
                    for d in range(DC):
                        nc.tensor.matmul(
                            ps,
                            hta[:, d, tcc * P : (tcc + 1) * P],
                            w_v[:, d, g * 8 : (g + 1) * 8, 0:DH],
                            start=(d == 0),
                            stop=(d == DC - 1),
                        )
                    nc.vector.tensor_copy(vt[:, tcc, g * 8 : (g + 1) * 8, 0:DH], ps)

            # ---- phase 2: per head ----
            def make_mk(h):
                mk = mkpool.tile([P, EC, TQ], f16, tag="mk", name=f"mk_{h}")
                for e in range(EC):
                    # split the broadcast multiplies across DVE and ACT
                    if e % 2 == 0:
                        nc.vector.tensor_tensor(
                            mk[:, e, :],
                            ktt[:, e, :],
                            mx[:, e * H + h, None].to_broadcast([P, TQ]),
                            mult,
                        )
                    else:
                        nc.scalar.activation(
                            mk[:, e, :], ktt[:, e, :], Copy, scale=mx[:, e * H + h, None]
                        )
                return mk

            def emit_scores(h, mk, sb):
                ets = []
                for tcc in range(TC):
                    ps = psm.tile([P, NB], f32, tag="m")
                    for e in range(EC):
                        nc.tensor.matmul(
                            ps,
                            mk[:, e, tcc * P : (tcc + 1) * P],
                            qt[:, e, sb * NB : (sb + 1) * NB],
                            start=(e == 0),
                            stop=(e == EC - 1),
                        )
                    et = epool.tile([P, NB], f16, tag="et", name=f"et_{h}_{sb}_{tcc}")
                    nc.scalar.activation(et, ps, Exp, scale=SCALE)
                    ets.append(et)
                return ets

            drain_engs = [nc.sync, nc.gpsimd, nc.scalar]

            def emit_ctx(h, sb, ets):
                # The program's makespan ends on the last output DMA's fixed
                # ~2.2us completion latency.  For the final two blocks, skip
                # the SBUF staging copy and DMA each s-chunk straight from
                # PSUM, spread across all three DMA-capable queues, so the
                # last DMA is issued as early as possible.
                tail = h == H - 1 and sb >= SB - 2
                ob = opool.tile([P, SC, CW], f32, tag="ob", name=f"ob_{h}_{sb}")
                for sc in range(SC):
                    cp = psc.tile([P, NB], f32, tag="c", name=f"cp_{h}_{sb}_{sc}")
                    cpu = cp[:, 0:CW]
                    for tcc in range(TC):
                        nc.tensor.matmul(
                            cpu,
                            ets[tcc][:, sc * P : (sc + 1) * P],
                            vt[:, tcc, h, :],
                            start=(tcc == 0),
                            stop=(tcc == TC - 1),
                        )
                    nc.vector.tensor_copy(ob[:, sc, :], cpu)
                    if tail:
                        eng = drain_engs[(sb * SC + sc) % 3]
                        eng.dma_start(ctx_view[h, sb, :, sc, :], ob[:, sc, :])
                if not tail:
                    eng = nc.sync if (h * SB + sb) % 2 == 0 else nc.gpsimd
                    eng.dma_start(ctx_view[h, sb], ob)

            # Software pipeline over the flat block stream (h0,sb0..3),(h1,..):
            # ctx of a block is emitted one block later, so the PE never waits
            # on the ACT exp of the block it just scored.
            mks = {0: make_mk(0), 1: make_mk(1)}
            pend = None
            for h in range(H):
                for sb in range(SB):
                    if sb == 0 and 1 <= h < H - 1:
                        mks[h + 1] = make_mk(h + 1)
                    ets = emit_scores(h, mks[h], sb)
                    if pend is not None:
                        emit_ctx(*pend)
                    pend = (h, sb, ets)
                mks.pop(h, None)
            emit_ctx(*pend)

    nc.compile()
    return nc


def make_in_maps(hidden_states, Wq, Wk, Wv, bv, mixing):
    """Host-side sharding: build per-core input dicts.

    Core c = (b, qt): batch b = c//4, t-quarter qt = c%4.  ht is rolled so the
    core's 512 keys sit at columns 0:512; all other inputs are identical on
    every core.
    """
    hidden_states = np.asarray(hidden_states, dtype=np.float32)
    Wq = np.asarray(Wq, dtype=np.float32)
    Wk = np.asarray(Wk, dtype=np.float32)
    Wv = np.asarray(Wv, dtype=np.float32)
    bv = np.asarray(bv, dtype=np.float32)
    mixing = np.asarray(mixing, dtype=np.float32)

    bf = np.float16
    wqt = np.ascontiguousarray(Wq.T).astype(bf)  # [d, e]
    wkt = np.ascontiguousarray(Wk.T).astype(bf)

    # Wv augmented: per head 64 v-columns then a denominator column that picks
    # out the ones row of ht; bias folded in via the same ones row.
    wvt = np.zeros((DC * P, H * CW), dtype=bf)
    wvT = Wv.T  # [d, dv]
    for h in range(H):
        wvt[:D, h * CW : h * CW + DH] = wvT[:, h * DH : (h + 1) * DH].astype(bf)
        wvt[D, h * CW : h * CW + DH] = bv[h * DH : (h + 1) * DH].astype(bf)
        wvt[D, h * CW + DH] = 1.0

    # mix[p, e*H + h] = mixing[h, e*128 + p]
    mx = np.ascontiguousarray(
        mixing.reshape(H, EC, P).transpose(2, 1, 0).reshape(P, EC * H)
    ).astype(np.float32)

    ht_by_b = []
    for b in range(B):
        ht = np.zeros((DC * P, S), dtype=bf)
        ht[:D] = hidden_states[b].T.astype(bf)
        ht[D] = 1.0
        ht_by_b.append(ht)

    in_maps = []
    for c in range(N_CORES):
        b, qt = divmod(c, 4)
        in_maps.append(
            {
                "ht": np.roll(ht_by_b[b], -TQ * qt, axis=1),
                "wqt": wqt,
                "wkt": wkt,
                "wvt": wvt,
                "mix": mx,
            }
        )
    return in_maps


def assemble_output(results):
    """results: list of per-core dicts with 'ctx' [H, SB, P, SC*CW] f32
    (rolled-s rows grouped (sb, sc, p), partial numerator cols 0:64 +
    partial denominator col 64)."""
    out = np.empty((B, S, DV), dtype=np.float32)
    for b in range(B):
        num = np.zeros((H, S, DH), dtype=np.float32)
        den = np.zeros((H, S, 1), dtype=np.float32)
        for qt in range(4):
            arr = results[b * 4 + qt]["ctx"].reshape(H, SB, P, SC, CW)
            arr = arr.transpose(0, 1, 3, 2, 4).reshape(H, S, CW)
            arr = np.roll(arr, TQ * qt, axis=1)
            num += arr[:, :, :DH]
            den += arr[:, :, DH:]
        out[b] = (num / den).transpose(1, 0, 2).reshape(S, DV)
    return out


def _get_runner():
    """Build (once) a jitted shard_map over the 8 cores running the compiled
    Bass program via the bass_exec custom call."""
    if "runner" in _CACHE:
        return _CACHE["runner"]

    import jax
    import concourse.mybir as mybir
    from jax.sharding import Mesh, PartitionSpec
    from jax.experimental.shard_map import shard_map
    from concourse import bass2jax
    from concourse.bass2jax import _bass_exec_p, partition_id_tensor

    bass2jax.install_neuronx_cc_hook()
    nc = _CACHE.setdefault("nc", build_program())

    part_name = nc.partition_id_tensor.name if nc.partition_id_tensor else None
    dbg_name = nc.dbg_addr.name if nc.dbg_addr is not None else None
    in_names, out_names, out_avals, zero_outs = [], [], [], []
    for alloc in nc.m.functions[0].allocations:
        if not isinstance(alloc, mybir.MemoryLocationSet):
            continue
        name = alloc.memorylocations[0].name
        if alloc.kind == "ExternalInput":
            if name != part_name:
                in_names.append(name)
        elif alloc.kind == "ExternalOutput":
            out_names.append(name)
            shape = tuple(alloc.tensor_shape)
            dtype = mybir.dt.np(alloc.dtype)
            out_avals.append(jax.core.ShapedArray(shape, dtype))
            zero_outs.append(np.zeros(shape, dtype))
    n_params = len(in_names)
    all_names = in_names + out_names + ([part_name] if part_name else [])

    def _body(*args):
        operands = list(args)
        if part_name is not None:
            operands.append(partition_id_tensor())
        outs = _bass_exec_p.bind(
            *operands,
            out_avals=tuple(out_avals),
            in_names=tuple(all_names),
            out_names=tuple(out_names),
            lowering_input_output_aliases=(),
            sim_require_finite=True,
            sim_require_nnan=True,
            nc=nc,
        )
        return tuple(outs)

    devices = jax.devices()[:N_CORES]
    mesh = Mesh(np.asarray(devices), ("core",))
    spec = PartitionSpec("core")
    sharded = jax.jit(
        shard_map(
            _body,
            mesh=mesh,
            in_specs=(spec,) * (n_params + len(out_names)),
            out_specs=(spec,) * len(out_names),
            check_rep=False,
        ),
        keep_unused=True,
    )
    concat_zero = [
        np.zeros((N_CORES * z.shape[0], *z.shape[1:]), z.dtype) for z in zero_outs
    ]

    def run(in_maps):
        def core_input(c, name):
            if name == dbg_name:
                return np.zeros((1, 2), np.uint32)
            return in_maps[c][name]

        concat_in = [
            np.concatenate([core_input(c, name) for c in range(N_CORES)], axis=0)
            for name in in_names
        ]
        out_arrs = sharded(*concat_in, *concat_zero)
        return [
            {
                name: np.asarray(out_arrs[i]).reshape(
                    N_CORES, *out_avals[i].shape
                )[c]
                for i, name in enumerate(out_names)
            }
            for c in range(N_CORES)
        ]

    _CACHE["runner"] = run
    return run


def kernel(hidden_states, Wq, Wk, Wv, bv, mixing):
    run = _get_runner()
    in_maps = make_in_maps(hidden_states, Wq, Wk, Wv, bv, mixing)
    return assemble_output(run(in_maps))
